# revision 19
# baseline (speedup 1.0000x reference)
"""Trainium2 Bass kernel for nn_DiffusionModel1d (batched tridiagonal solve).

Math: the reference solves A(K) u = h^2 * f with K = exp(alpha) and
A = tridiag(-K_j, K_j + K_{j+1}, -K_{j+1})  (row 0 diag 2K_0 + K_1,
row n-1 = K_{n-1} (u_{n-1} - u_{n-2})).  This matrix factors exactly as

    A = B^T diag(K_0..K_{n-1}) B + K_0 e_0 e_0^T

with B the unit lower-bidiagonal first-difference operator, so by
Sherman-Morrison (and since B^{-1} is cumsum, B^{-T} rev-cumsum):

    z = cumsum( rev_cumsum(h^2 f) / K );   u = z - z_0 / 2

Folding the -z_0/2 correction into a halved first weight gives

    u = cumsum( w' * exp(-alpha) ),  w' = rev_cumsum(h^2 f), w'_0 *= 0.5

i.e. one exp, one multiply and one hardware prefix-scan per element.
Sharding: pure data parallel over batch across the 8 cores.

Engine balance per 128-row tile: SP issues the load and (two tiles
behind) the store; ACT computes exp(-alpha); the multiply runs on DVE
for every 4th tile and on Pool otherwise; DVE runs the prefix scan.
Every engine stays under ~25 us per core so the kernel sits on the
HBM/DMA roofline (~16.7 MB per core of unavoidable traffic).

Written in raw Bass (explicit semaphores, standalone wait instructions):
the walrus build in this container rejects any compute instruction whose
sync_info carries more than one semaphore wait, which rules out
Tile-generated scheduling here.  DMA completion uses one semaphore per
pipeline buffer slot: consecutive DMAs from one engine land on different
queues and complete out of order, so a cumulative counter cannot
identify one transfer; slot semaphores are sound because the next DMA
on a slot only issues after its consumer drained the previous one.
"""

import numpy as np

import concourse.bass as bass
import concourse.mybir as mybir
from concourse.bass_utils import run_bass_kernel_spmd

F32 = mybir.dt.float32
ALU = mybir.AluOpType
ACT_EXP = mybir.ActivationFunctionType.Exp

N_CORES = 8
B_FULL = 16384
M = 1024
N = M - 1               # 1023 unknowns
P = 128                 # SBUF partitions
B_SHARD = B_FULL // N_CORES
NB = 8                  # pipeline buffers per stage
H2 = 1.0 / float(N * N)
STORE_LAG = 2           # store(t) issued after load(t+STORE_LAG) on SP
N_SETUP = 5             # DVE ops in the W' setup chain


def _build(b_shard: int = B_SHARD, rep: int = 1,
           dve_mult_stride: int = 1, store_on: str = "pool",
           act_load_stride: int = 0, store_act_stride: int = 0) -> bass.Bass:
    """Build the kernel.

    rep > 1 repeats the whole per-core pipeline rep times inside one
    NEFF (same data, same output) — used by test.py to measure pure HW
    kernel time as a wall-clock slope over rep.
    dve_mult_stride k: the multiply of every k-th tile runs on DVE, the
    rest on Pool (k=1: all DVE, k=0: all Pool).
    store_on: "pool" issues output DMAs through the gpsimd SWDGE (its 8
    SW queues are the only way to engage the second half of the 16 DMA
    engines; HWDGE loads alone cap at ~190 GB/s), "sp" uses SP HWDGE.
    act_load_stride k > 0: every k-th load is issued from the ACT HWDGE
    instead of SP (probe for extra HWDGE queue parallelism).
    store_act_stride k > 0: every k-th store is issued from the ACT
    HWDGE instead of the store_on engine.
    """
    nc = bass.Bass("TRN2")
    alpha = nc.dram_tensor("alpha", [b_shard, M], F32, kind="ExternalInput")
    f_rhs = nc.dram_tensor("f_rhs", [N], F32, kind="ExternalInput")
    out = nc.dram_tensor("u", [b_shard, N], F32, kind="ExternalOutput")

    ntiles = b_shard // P
    nt = rep * ntiles
    alpha_t = alpha[:].rearrange("(t p) m -> t p m", p=P)
    out_t = out[:].rearrange("(t p) m -> t p m", p=P)

    def on_dve(t):
        return dve_mult_stride > 0 and t % dve_mult_stride == 0

    # cumulative per-engine multiply counts: value of the engine's mult
    # semaphore after mult(t) completes
    def mult_cnt(t):
        n_dve = sum(1 for q in range(t + 1) if on_dve(q))
        return n_dve if on_dve(t) else (t + 1 - n_dve)

    # DRAM view of f broadcast across all 128 partitions
    f_ap = f_rhs[:][None, :]
    f_bcast = bass.AP(tensor=f_ap.tensor, offset=f_ap.offset,
                      ap=[[0, P]] + [list(d) for d in f_ap.ap[1:]])

    import contextlib
    with contextlib.ExitStack() as ctx:
        fb = ctx.enter_context(nc.sbuf_tensor([P, N], F32))    # f -> g = h2*f
        csum = ctx.enter_context(nc.sbuf_tensor([P, N], F32))  # cumsum of g
        w = ctx.enter_context(nc.sbuf_tensor([P, N], F32))     # weight row
        A = ctx.enter_context(nc.sbuf_tensor([P, NB, M], F32))  # alpha tiles
        R = ctx.enter_context(nc.sbuf_tensor([P, NB, N], F32))  # exp(-alpha)
        X = ctx.enter_context(nc.sbuf_tensor([P, NB, N], F32))  # w * R
        U = ctx.enter_context(nc.sbuf_tensor([P, NB, N], F32))  # cumsum(X)
        s_fw = ctx.enter_context(nc.semaphore("s_fw"))
        s_load = [ctx.enter_context(nc.semaphore(f"s_load{i}"))
                  for i in range(NB)]
        s_store = [ctx.enter_context(nc.semaphore(f"s_store{i}"))
                   for i in range(NB)]
        s_act = ctx.enter_context(nc.semaphore("s_act"))
        s_w = ctx.enter_context(nc.semaphore("s_w"))
        s_mult_dve = ctx.enter_context(nc.semaphore("s_mult_dve"))
        s_mult_pool = ctx.enter_context(nc.semaphore("s_mult_pool"))
        s_scan = ctx.enter_context(nc.semaphore("s_scan"))
        block = ctx.enter_context(nc.Block())

        def emit_store(eng, t):
            b = t % NB
            eng.wait_ge(s_scan, t + 1)
            eng.dma_start(out=out_t[t % ntiles],
                          in_=U[:, b, :]).then_inc(s_store[b], 16)

        def on_act_load(t):
            return act_load_stride > 0 and t % act_load_stride == 0

        def on_act_store(t):
            return store_act_stride > 0 and t % store_act_stride == 0

        def emit_load(eng, t):
            b = t % NB
            if t >= NB:
                # A[:, b] is free once exp(t-NB) consumed it
                eng.wait_ge(s_act, t - NB + 1)
            eng.dma_start(out=A[:, b, :],
                          in_=alpha_t[t % ntiles, :, :]).then_inc(
                              s_load[b], 16)

        @block.sync
        def _(sync):
            # one-time: fetch f (replicated to all partitions)
            sync.dma_start(out=fb[:, :], in_=f_bcast).then_inc(s_fw, 16)
            for t in range(nt):
                if not on_act_load(t):
                    emit_load(sync, t)
                if store_on == "sp":
                    if t >= STORE_LAG and not on_act_store(t - STORE_LAG):
                        emit_store(sync, t - STORE_LAG)
            if store_on == "sp":
                for t in range(nt - STORE_LAG, nt):
                    if not on_act_store(t):
                        emit_store(sync, t)

        @block.scalar
        def _(scalar):
            for t in range(nt):
                b = t % NB
                if on_act_load(t):
                    emit_load(scalar, t)
                if t >= STORE_LAG and on_act_store(t - STORE_LAG):
                    emit_store(scalar, t - STORE_LAG)
                scalar.wait_ge(s_load[b], 16 * (t // NB + 1))
                if t >= NB:
                    # R[:, b] is free once mult(t-NB) consumed it
                    tm = t - NB
                    scalar.wait_ge(
                        s_mult_dve if on_dve(tm) else s_mult_pool,
                        mult_cnt(tm))
                nc.scalar.activation(R[:, b, :], A[:, b, 0:N], ACT_EXP,
                                     scale=-1.0).then_inc(s_act, 1)
            for t in range(nt - STORE_LAG, nt):
                if on_act_store(t):
                    emit_store(scalar, t)

        def emit_mult(eng_wait, engine, t):
            b = t % NB
            eng_wait(s_act, t + 1)
            eng_wait(s_w, N_SETUP)
            if t >= NB:
                # X[:, b] is free once scan(t-NB) consumed it
                eng_wait(s_scan, t - NB + 1)
            sem = s_mult_dve if on_dve(t) else s_mult_pool
            engine.tensor_tensor(X[:, b, :], R[:, b, :], w[:, :],
                                 ALU.mult).then_inc(sem, 1)

        @block.vector
        def _(vector):
            # ---- one-time W' setup (5 chained DVE ops, s_w = 1..5) ----
            vector.wait_ge(s_fw, 16)
            nc.vector.tensor_scalar_mul(fb[:, :], fb[:, :],
                                        float(H2)).then_inc(s_w, 1)
            vector.wait_ge(s_w, 1)
            nc.vector.tensor_tensor_scan(csum[:, :], fb[:, :], fb[:, :], 0.0,
                                         ALU.add, ALU.bypass).then_inc(s_w, 1)
            vector.wait_ge(s_w, 2)
            # w = (-csum + g) + csum[:, -1]  == rev_cumsum(g)
            nc.vector.scalar_tensor_tensor(w[:, :], csum[:, :], -1.0, fb[:, :],
                                           ALU.mult, ALU.add).then_inc(s_w, 1)
            vector.wait_ge(s_w, 3)
            nc.vector.tensor_scalar_add(w[:, :], w[:, :],
                                        csum[:, N - 1:N]).then_inc(s_w, 1)
            vector.wait_ge(s_w, 4)
            nc.vector.tensor_scalar_mul(w[:, 0:1], w[:, 0:1],
                                        0.5).then_inc(s_w, 1)
            # ---- per-tile: multiply (every k-th tile) + prefix scan ----
            for t in range(nt):
                b = t % NB
                if on_dve(t):
                    emit_mult(vector.wait_ge, nc.vector, t)
                vector.wait_ge(s_mult_dve if on_dve(t) else s_mult_pool,
                               mult_cnt(t))
                if t >= NB:
                    # U[:, b] is free once store(t-NB) finished
                    vector.wait_ge(s_store[b], 16 * ((t - NB) // NB + 1))
                nc.vector.tensor_tensor_scan(U[:, b, :], X[:, b, :],
                                             X[:, b, :], 0.0, ALU.add,
                                             ALU.bypass).then_inc(s_scan, 1)

        @block.gpsimd
        def _(gpsimd):
            for t in range(nt):
                if not on_dve(t):
                    emit_mult(gpsimd.wait_ge, nc.gpsimd, t)
                if (store_on == "pool" and t >= STORE_LAG
                        and not on_act_store(t - STORE_LAG)):
                    emit_store(gpsimd, t - STORE_LAG)
            if store_on == "pool":
                for t in range(nt - STORE_LAG, nt):
                    if not on_act_store(t):
                        emit_store(gpsimd, t)
        # tail stores assigned to ACT are emitted in the scalar block only
        # for t-STORE_LAG < nt-STORE_LAG; emit the remaining ones there too

    return nc


_cache: dict = {}


def kernel(alpha: np.ndarray, f_rhs: np.ndarray) -> np.ndarray:
    assert alpha.shape == (B_FULL, M) and f_rhs.shape == (N,)
    if "nc" not in _cache:
        _cache["nc"] = _build()
    nc = _cache["nc"]

    f32 = np.ascontiguousarray(f_rhs, dtype=np.float32)
    in_maps = [
        {
            "alpha": np.ascontiguousarray(alpha[i * B_SHARD:(i + 1) * B_SHARD]),
            "f_rhs": f32,
        }
        for i in range(N_CORES)
    ]
    # The axon-tunneled devices occasionally come up wedged from a prior
    # aborted process and fail the first dispatch with
    # NRT_EXEC_UNIT_UNRECOVERABLE; the condition self-heals, so retry.
    last_exc = None
    for _ in range(3):
        try:
            res = run_bass_kernel_spmd(nc, in_maps, list(range(N_CORES)))
            break
        except Exception as exc:  # noqa: BLE001
            last_exc = exc
            import time as _time
            _time.sleep(5)
            try:
                import jax
                jax.clear_caches()
                jax.clear_backends()
            except Exception:  # noqa: BLE001
                pass
    else:
        raise last_exc
    return np.concatenate([res.results[i]["u"] for i in range(N_CORES)], axis=0)
